# revision 8
# baseline (speedup 1.0000x reference)
"""Trainium2 Bass kernel for the composite LM-CE + detection-matching loss.

Contract: kernel(**inputs) takes the FULL unsharded inputs (numpy arrays,
keyed as in setup_inputs()) and returns the FULL scalar loss.

Sharding (8 cores, SPMD single program):
  - LM cross-entropy: the B*S = 2048 token rows are split 256/core. Each
    core streams its [256, 32000] f32 logit shard from HBM once (the
    memory-bound part), computing sum(exp(row)) via ACT Exp+accum, then
    lse = ln(S).  x[label] comes from an indirect-DMA gather using
    host-computed flat element indices.  Each core emits
    sum(mask*(lse - x[label])) as a partial.
  - Detection loss: core i processes image i % 2 (B == 2); the host reads
    det partials from cores 0 and 1 only.  The greedy IoU matching loop is
    done with an equality-mask formulation (no data-dependent control
    flow): per step find the global max of the masked [100, 25] IoU
    matrix, gate by >= 0.5, accumulate the matched pair loss from a
    precomputed pairwise GIoU+SmoothL1 matrix, and add NEG to the matched
    row and column.
  - Host combines the 8 partial sums (the gather step) into the scalar.
"""

import os
from contextlib import ExitStack

import numpy as np

import concourse.bacc as bacc
import concourse.tile as tile
from concourse import mybir
from concourse.bass import IndirectOffsetOnAxis
from concourse.bass_utils import run_bass_kernel_spmd
from concourse.masks import make_identity

# problem constants (hardcoded; kernel.py must be self-contained)
B, S, V = 2, 1024, 32000
N, M, C = 100, 25, 80
CLS_W, COORD_W = 0.0, 0.7
IOU_W, L1_W = 0.75, 0.25
LM_W, DET_W = 0.2, 0.8
EPS = 1e-7
NEG = -1e9
PEN = 0.2 * COORD_W * L1_W + 0.2 * CLS_W  # 0.035

NCORES = 8
ROWS = B * S          # 2048
RPC = ROWS // NCORES  # 256 rows per core
RT = RPC // 128       # 2 row-tiles of 128 rows
VC = 8000             # vocab chunk (32KB/partition per tile)
NCH = V // VC         # 4 chunks per row-tile

F32 = mybir.dt.float32
I32 = mybir.dt.int32
X = mybir.AxisListType.X
OP = mybir.AluOpType
AF = mybir.ActivationFunctionType

_CACHE = {}


def _build_program():
    nc = bacc.Bacc("TRN2", target_bir_lowering=False, debug=False)

    lm = nc.dram_tensor("lm", [RPC * V, 1], F32, kind="ExternalInput")
    gidx = nc.dram_tensor("gidx", [RPC, 1], I32, kind="ExternalInput")
    msk = nc.dram_tensor("msk", [RPC, 1], F32, kind="ExternalInput")
    pb = nc.dram_tensor("pb", [N, 4], F32, kind="ExternalInput")
    # tbt row layout: x(25) y(25) w(25) h(25)
    tbt = nc.dram_tensor("tbt", [1, 4 * M], F32, kind="ExternalInput")
    lv = nc.dram_tensor("lv", [1, M], F32, kind="ExternalInput")
    outd = nc.dram_tensor("out", [1, 8], F32, kind="ExternalOutput")

    with tile.TileContext(nc) as tc, ExitStack() as ctx:
        _body(ctx, tc, nc, lm, gidx, msk, pb, tbt, lv, outd)
    nc.compile()
    return nc


def _body(ctx, tc, nc, lm, gidx, msk, pb, tbt, lv, outd):
    lm2d = lm[:].rearrange("(r v) o -> r (v o)", r=RPC)  # [256, 32000]

    const = ctx.enter_context(tc.tile_pool(name="const", bufs=1))
    data = ctx.enter_context(tc.tile_pool(name="data", bufs=3))
    scr = ctx.enter_context(tc.tile_pool(name="scr", bufs=1))
    small = ctx.enter_context(tc.tile_pool(name="small", bufs=1))
    dloop = ctx.enter_context(tc.tile_pool(name="dloop", bufs=2))
    psum = ctx.enter_context(tc.tile_pool(name="psum", bufs=1, space="PSUM"))

    tt = nc.vector.tensor_tensor
    ts = nc.vector.tensor_scalar
    stt = nc.vector.scalar_tensor_tensor

    # ---------------- constants ----------------
    ones_p = const.tile([128, 1], F32)
    nc.vector.memset(ones_p[:], 1.0)
    ones_f = const.tile([1, 128], F32)
    nc.vector.memset(ones_f[:], 1.0)
    negc = const.tile([N, M], F32)
    nc.vector.memset(negc[:], NEG)
    ident = const.tile([128, 128], F32)
    make_identity(nc, ident[:])
    negone = const.tile([128, 1], F32)
    nc.vector.memset(negone[:], -1.0)

    # ---------------- LM: label gathers (early, overlap the big stream) ----
    xls = []
    mskt = small.tile([128, RT], F32)
    for t in range(RT):
        gi = small.tile([128, 1], I32, tag=f"gi{t}")
        nc.sync.dma_start(gi[:], gidx[t * 128:(t + 1) * 128, :])
        xl = small.tile([128, 1], F32, tag=f"xl{t}")
        nc.gpsimd.indirect_dma_start(
            out=xl[:],
            out_offset=None,
            in_=lm[:, :],
            in_offset=IndirectOffsetOnAxis(ap=gi[:, :1], axis=0),
        )
        xls.append(xl)
        nc.sync.dma_start(mskt[:, t:t + 1], msk[t * 128:(t + 1) * 128, :])

    # ---------------- LM: stream chunks, exp + row-accumulate on ACT -------
    sums = small.tile([128, RT * NCH], F32)
    for t in range(RT):
        for c in range(NCH):
            dtile = data.tile([128, VC], F32, tag="d")
            nc.sync.dma_start(
                dtile[:], lm2d[t * 128:(t + 1) * 128, c * VC:(c + 1) * VC]
            )
            es = scr.tile([128, VC], F32, tag="es")
            col = t * NCH + c
            nc.scalar.activation(
                es[:], dtile[:], AF.Exp, accum_out=sums[:, col:col + 1]
            )

    # nlldet col0: per-partition LM nll partial; col1: det matched-loss partial
    nlldet = small.tile([128, 2], F32)
    nc.vector.memset(nlldet[:], 0.0)

    Srow = small.tile([128, RT], F32)
    for t in range(RT):
        nc.vector.reduce_sum(
            Srow[:, t:t + 1], sums[:, t * NCH:(t + 1) * NCH], axis=X
        )
    logS = small.tile([128, RT], F32)
    nc.scalar.activation(logS[:], Srow[:], AF.Ln)
    nllv = small.tile([128, RT], F32)
    for t in range(RT):
        # (lse - x[label]) * mask
        stt(
            out=nllv[:, t:t + 1], in0=logS[:, t:t + 1], scalar=xls[t][:, :1],
            in1=mskt[:, t:t + 1], op0=OP.subtract, op1=OP.mult,
        )
    nc.vector.reduce_sum(nlldet[:, 0:1], nllv[:], axis=X)

    # ---------------- DET: load per-image tensors --------------------------
    pbt = small.tile([N, 4], F32)
    nc.sync.dma_start(pbt[:], pb[:, :])
    tbs = small.tile([1, 4 * M], F32)
    nc.sync.dma_start(tbs[:], tbt[:, :])
    lvs = small.tile([1, M], F32)
    nc.sync.dma_start(lvs[:], lv[:, :])

    # pred corners: x2y2 = xy + wh ; area_a from corners (matches reference)
    pxy2 = small.tile([N, 2], F32)
    tt(pxy2[:], pbt[:, 0:2], pbt[:, 2:4], op=OP.add)
    wA = small.tile([N, 2], F32)
    tt(wA[:], pxy2[:], pbt[:, 0:2], op=OP.subtract)
    areaA = small.tile([N, 1], F32)
    tt(areaA[:], wA[:, 0:1], wA[:, 1:2], op=OP.mult)
    px1 = pbt[:, 0:1]
    py1 = pbt[:, 1:2]
    px2 = pxy2[:, 0:1]
    py2 = pxy2[:, 1:2]

    # target row: [x1(25) y1(25) x2(25) y2(25) validNEG(25) areaB(25)]
    trow = small.tile([1, 6 * M], F32)
    nc.vector.tensor_copy(trow[:, 0:2 * M], tbs[:, 0:2 * M])
    tt(trow[:, 2 * M:4 * M], tbs[:, 0:2 * M], tbs[:, 2 * M:4 * M], op=OP.add)
    # valid = (w>0)*(h>0)*labelvalid
    v1 = small.tile([1, 2 * M], F32)
    ts(v1[:], tbs[:, 2 * M:4 * M], 0.0, None, op0=OP.is_gt)
    v3 = small.tile([1, M], F32)
    tt(v3[:], v1[:, 0:M], v1[:, M:2 * M], op=OP.mult)
    tt(v3[:], v3[:], lvs[:], op=OP.mult)
    # validNEG = (v-1)*1e9 = v*(-NEG) + NEG
    ts(trow[:, 4 * M:5 * M], v3[:], -NEG, NEG, op0=OP.mult, op1=OP.add)
    # areaB from target corners (exact reference arithmetic)
    tw = small.tile([1, 2 * M], F32)
    tt(tw[:], trow[:, 2 * M:4 * M], trow[:, 0:2 * M], op=OP.subtract)
    tt(trow[:, 5 * M:6 * M], tw[:, 0:M], tw[:, M:2 * M], op=OP.mult)
    nv = small.tile([1, 1], F32)
    nc.vector.reduce_sum(nv[:], v3[:], axis=X)

    # broadcast target row across 100 partitions via PE
    tcb = psum.tile([N, 6 * M], F32, tag="tcb")
    nc.tensor.matmul(
        out=tcb[:], lhsT=ones_f[0:1, 0:N], rhs=trow[:], start=True, stop=True
    )
    TX1 = tcb[:, 0 * M:1 * M]
    TY1 = tcb[:, 1 * M:2 * M]
    TX2 = tcb[:, 2 * M:3 * M]
    TY2 = tcb[:, 3 * M:4 * M]
    VNEG = tcb[:, 4 * M:5 * M]
    AB = tcb[:, 5 * M:6 * M]

    def pairwise(name):
        return small.tile([N, M], F32, tag=name, name=name)

    # intersection
    ltx = pairwise("ltx")
    ts(ltx[:], TX1, px1, None, op0=OP.max)
    lty = pairwise("lty")
    ts(lty[:], TY1, py1, None, op0=OP.max)
    rbx = pairwise("rbx")
    ts(rbx[:], TX2, px2, None, op0=OP.min)
    rby = pairwise("rby")
    ts(rby[:], TY2, py2, None, op0=OP.min)
    iw = pairwise("iw")
    tt(iw[:], rbx[:], ltx[:], op=OP.subtract)
    ts(iw[:], iw[:], 0.0, None, op0=OP.max)
    ih = pairwise("ih")
    tt(ih[:], rby[:], lty[:], op=OP.subtract)
    ts(ih[:], ih[:], 0.0, None, op0=OP.max)
    inter = pairwise("inter")
    tt(inter[:], iw[:], ih[:], op=OP.mult)
    # union = (areaB + areaA) - inter
    union = pairwise("union")
    stt(out=union[:], in0=AB, scalar=areaA[:, :1], in1=inter[:],
        op0=OP.add, op1=OP.subtract)
    # matching iou = inter / max(union, EPS)  (+ NEG on invalid cols)
    um = pairwise("um")
    ts(um[:], union[:], EPS, None, op0=OP.max)
    nc.vector.reciprocal(um[:], um[:])
    iou = small.tile([N, M], F32)       # persistent matching matrix
    tt(iou[:], inter[:], um[:], op=OP.mult)
    tt(iou[:], iou[:], VNEG, op=OP.add)
    # giou-loss iou' = inter / (union + EPS)
    ue = pairwise("ue")
    ts(ue[:], union[:], EPS, None, op0=OP.add)
    nc.vector.reciprocal(ue[:], ue[:])
    iouG = pairwise("iouG")
    tt(iouG[:], inter[:], ue[:], op=OP.mult)
    # enclosing box
    cltx = pairwise("cltx")
    ts(cltx[:], TX1, px1, None, op0=OP.min)
    clty = pairwise("clty")
    ts(clty[:], TY1, py1, None, op0=OP.min)
    crbx = pairwise("crbx")
    ts(crbx[:], TX2, px2, None, op0=OP.max)
    crby = pairwise("crby")
    ts(crby[:], TY2, py2, None, op0=OP.max)
    cw = pairwise("cw")
    tt(cw[:], crbx[:], cltx[:], op=OP.subtract)
    ts(cw[:], cw[:], 0.0, None, op0=OP.max)
    ch = pairwise("ch")
    tt(ch[:], crby[:], clty[:], op=OP.subtract)
    ts(ch[:], ch[:], 0.0, None, op0=OP.max)
    areaC = pairwise("areaC")
    tt(areaC[:], cw[:], ch[:], op=OP.mult)
    acmu = pairwise("acmu")
    tt(acmu[:], areaC[:], union[:], op=OP.subtract)
    ace = pairwise("ace")
    ts(ace[:], areaC[:], EPS, None, op0=OP.add)
    nc.vector.reciprocal(ace[:], ace[:])
    frac = pairwise("frac")
    tt(frac[:], acmu[:], ace[:], op=OP.mult)
    # gl = 1 - (iouG - frac)
    gl = pairwise("gl")
    tt(gl[:], iouG[:], frac[:], op=OP.subtract)
    ts(gl[:], gl[:], -1.0, 1.0, op0=OP.mult, op1=OP.add)
    # smooth l1 over the 4 corner coords
    slsum = pairwise("slsum")
    nc.vector.memset(slsum[:], 0.0)
    for ci, (tc_ap, pc_ap) in enumerate(
        [(TX1, px1), (TY1, py1), (TX2, px2), (TY2, py2)]
    ):
        d = pairwise("sl_d")
        ts(d[:], tc_ap, pc_ap, None, op0=OP.subtract)
        a = pairwise("sl_a")
        nc.scalar.activation(a[:], d[:], AF.Abs)
        m_ = pairwise("sl_m")
        ts(m_[:], a[:], 1.0, None, op0=OP.min)
        sq = pairwise("sl_sq")
        nc.scalar.activation(sq[:], m_[:], AF.Square, scale=float(np.sqrt(0.5)))
        r = pairwise("sl_r")
        nc.scalar.activation(r[:], a[:], AF.Relu, bias=negone[0:N, 0:1])
        tt(sq[:], sq[:], r[:], op=OP.add)
        tt(slsum[:], slsum[:], sq[:], op=OP.add)
    # L = COORD_W*(IOU_W*gl + L1_W*mean4(sl)) = 0.525*gl + 0.04375*slsum
    Lmat = small.tile([N, M], F32)
    glw = pairwise("glw")
    ts(glw[:], gl[:], COORD_W * IOU_W, None, op0=OP.mult)
    stt(out=Lmat[:], in0=slsum[:], scalar=COORD_W * L1_W * 0.25, in1=glw[:],
        op0=OP.mult, op1=OP.add)

    # ---------------- DET: greedy matching, 25 equality-mask steps ---------
    nmacc = small.tile([1, 1], F32)
    nc.vector.memset(nmacc[:], 0.0)

    for _k in range(M):
        rmax = dloop.tile([128, 1], F32, tag="rmax")
        nc.vector.reduce_max(rmax[0:N], iou[:], axis=X)
        trp = psum.tile([1, 128], F32, tag="trp")
        nc.tensor.transpose(
            out=trp[0:1, 0:N], in_=rmax[0:N], identity=ident[0:N, 0:N]
        )
        gms = dloop.tile([1, 1], F32, tag="gms")
        nc.vector.reduce_max(gms[:], trp[0:1, 0:N], axis=X)
        gbp = psum.tile([128, 1], F32, tag="gbp")
        nc.tensor.matmul(
            out=gbp[0:N], lhsT=ones_f[0:1, 0:N], rhs=gms[:], start=True,
            stop=True,
        )
        # eqn = (iou >= gmax) * NEG   (exactly one cell, except degenerate
        # all-tied cases where every tied cell is masked at once; those only
        # occur when gmax < 0.5 so the loss contribution is 0 either way)
        eqn = dloop.tile([128, M], F32, tag="eqn")
        stt(out=eqn[0:N], in0=iou[:], scalar=gbp[0:N, 0:1], in1=negc[:],
            op0=OP.is_ge, op1=OP.mult)
        # ok gate scaled by 1/NEG to cancel eqn's NEG factor
        oks = dloop.tile([128, 1], F32, tag="oks")
        ts(oks[0:N], gbp[0:N], 0.5, 1.0 / NEG, op0=OP.is_ge, op1=OP.mult)
        okb = dloop.tile([1, 1], F32, tag="okb")
        ts(okb[:], gbp[0:1, 0:1], 0.5, None, op0=OP.is_ge)
        tt(nmacc[:], nmacc[:], okb[:], op=OP.add)
        # matched pair loss: plok[p] = ok * sum_t eq[p,t] * L[p,t]
        tmp = dloop.tile([128, M], F32, tag="tmp")
        plok = dloop.tile([128, 1], F32, tag="plok")
        stt(out=tmp[0:N], in0=eqn[0:N], scalar=oks[0:N, 0:1], in1=Lmat[:],
            op0=OP.mult, op1=OP.mult, accum_out=plok[0:N, 0:1])
        tt(nlldet[0:N, 1:2], nlldet[0:N, 1:2], plok[0:N], op=OP.add)
        # row mask: NEG where rowmax == gmax
        rmn = dloop.tile([128, 1], F32, tag="rmn")
        stt(out=rmn[0:N], in0=rmax[0:N], scalar=gbp[0:N, 0:1],
            in1=negc[:, 0:1], op0=OP.is_ge, op1=OP.mult)
        # column mask: colsum(eqn) broadcast back over partitions
        cols = psum.tile([1, M], F32, tag="cols")
        nc.tensor.matmul(
            out=cols[:], lhsT=ones_p[0:N, 0:1], rhs=eqn[0:N], start=True,
            stop=True,
        )
        colsb = dloop.tile([1, M], F32, tag="colsb")
        nc.vector.tensor_copy(colsb[:], cols[:])
        colb = psum.tile([128, M], F32, tag="colb")
        nc.tensor.matmul(
            out=colb[0:N], lhsT=ones_f[0:1, 0:N], rhs=colsb[:], start=True,
            stop=True,
        )
        # iou += rowNEG (bcast over free) + colNEG
        stt(out=iou[:], in0=iou[:], scalar=rmn[0:N, 0:1], in1=colb[0:N],
            op0=OP.add, op1=OP.add)

    # ---------------- final partial sums -----------------------------------
    res = psum.tile([1, 2], F32, tag="res")
    nc.tensor.matmul(
        out=res[:], lhsT=ones_p[:], rhs=nlldet[:], start=True, stop=True
    )
    outsb = small.tile([1, 8], F32)
    nc.vector.memset(outsb[:], 0.0)
    nc.vector.tensor_copy(outsb[:, 0:2], res[:])
    nc.vector.tensor_copy(outsb[:, 2:3], nmacc[:])
    nc.vector.tensor_copy(outsb[:, 3:4], nv[:])
    nc.sync.dma_start(outd[:, :], outsb[:])


def _get_program():
    if "nc" not in _CACHE:
        _CACHE["nc"] = _build_program()
    return _CACHE["nc"]


def _prepare_in_maps(lm_logits, lm_labels, box_preds, target_labels,
                     target_boxes):
    lm_logits = np.ascontiguousarray(np.asarray(lm_logits, dtype=np.float32))
    box_preds = np.asarray(box_preds, dtype=np.float32)
    target_boxes = np.asarray(target_boxes, dtype=np.float32)

    lab_flat = np.asarray(lm_labels, dtype=np.int64).reshape(ROWS)
    lm_flat = lm_logits.reshape(ROWS, V)
    clipped = np.clip(lab_flat, 0, V - 1).astype(np.int64)
    mask_flat = (lab_flat != -100).astype(np.float32)
    total_cnt = float(max(mask_flat.sum(), 1.0))

    in_maps = []
    for i in range(NCORES):
        r0 = i * RPC
        img = i % B
        gi = (np.arange(RPC, dtype=np.int64) * V + clipped[r0:r0 + RPC]
              ).astype(np.int32).reshape(RPC, 1)
        tb = target_boxes[img]  # [25, 4] xywh
        tbt = np.ascontiguousarray(tb.T).reshape(1, 4 * M).astype(np.float32)
        lvv = (np.asarray(target_labels[img], dtype=np.int64) != -100
               ).astype(np.float32).reshape(1, M)
        in_maps.append({
            "lm": lm_flat[r0:r0 + RPC].reshape(RPC * V, 1),
            "gidx": gi,
            "msk": mask_flat[r0:r0 + RPC].reshape(RPC, 1).astype(np.float32),
            "pb": np.ascontiguousarray(box_preds[img]),
            "tbt": tbt,
            "lv": lvv,
        })
    return in_maps, total_cnt


def _combine(outs, total_cnt):
    nll_total = float(sum(o[0] for o in outs))
    lm_loss = nll_total / total_cnt
    det = []
    for img in range(B):
        o = outs[img]  # core `img` processed image `img`
        matched, nmatch, nvalid = float(o[1]), float(o[2]), float(o[3])
        unmatched = (N - nmatch) + (nvalid - nmatch)
        det.append(matched + PEN * unmatched)
    det_loss = sum(det) / B
    return np.float32(LM_W * lm_loss + DET_W * det_loss)


def kernel(
    lm_logits, lm_labels, class_logits, box_preds, target_labels,
    target_boxes, **_unused,
):
    nc = _get_program()
    in_maps, total_cnt = _prepare_in_maps(
        lm_logits, lm_labels, box_preds, target_labels, target_boxes
    )
    trace = bool(int(os.environ.get("KERNEL_TRACE", "0")))
    br = run_bass_kernel_spmd(
        nc, in_maps, core_ids=list(range(NCORES)), trace=trace
    )
    _CACHE["last_result"] = br
    outs = [np.asarray(br.results[i]["out"]).reshape(8) for i in range(NCORES)]
    return _combine(outs, total_cnt)
